# revision 29
# baseline (speedup 1.0000x reference)
"""GCN-GRU encoder (DCRNN-style) on 8 TRN2 NeuronCores, data-parallel over B.

v2: bf16 datapath + wide pair layout. Per core (B_loc=4 = 2 pairs):
  HW[p]   [64, 1024] bf16  GRU state, cols = b*512 + node (wide pair)
  HN_s[p] [128, 512] bf16  node-major h, col = j*128 + b*64 + f (agg lhsT)
  CN_s[p] same for r*h
  gt[k][j] [128, 512] bf16 G[k].T j-tile (agg rhs)
  t01[b]  [128, 512] bf16  per-batch aggregates rows (k0 f | k1 f)
  AXG/AXU_all[b] [70, T*512] bf16: rows 0:64 = k2 aggregate (written per
      step at col t*512), rows 64:70 = static x-aggregates (k,c) for all t
      -> the k2 weight matmul also carries the x-part (contraction 70).
Per batch per gcn only 2 weight matmuls ([k0|k1] @ 128, [k2|x] @ 70).
Gate zr [128,512] PSUM -> ONE sigmoid per batch (bias [128,1] = bz|br).
GRU elementwise in wide bf16: rh, d=hc-h, zd=z*d, h'=h+zd (DVE 2x mode).
"""
import numpy as np
import ml_dtypes

import concourse.bass as bass
import concourse.tile as tile
from concourse import mybir
from concourse.bass_utils import run_bass_kernel_spmd

dt = mybir.dt
AF = mybir.ActivationFunctionType
ALU = mybir.AluOpType

B, T, N, C, H, K = 32, 12, 512, 2, 64, 3
NCORES = 8
BL = B // NCORES          # 4 batches per core
NT = N // 128             # 4 partition tiles of the node dim
P = C + H                 # 66
BF = dt.bfloat16
NPBF = ml_dtypes.bfloat16

_waitsplit_ctr = [0]


def _split_excess_waits(nc, max_waits=1):
    """This walrus build allows only `max_waits` semaphore waits per
    instruction; hoist the excess onto preceding same-engine NoOps."""
    for f in nc.m.functions:
        for blk in f.blocks:
            new = []
            for inst in blk.instructions:
                si = inst.sync_info
                if si is not None and len(si.on_wait) > max_waits:
                    waits = list(si.on_wait)
                    head, tail = waits[:-max_waits], waits[-max_waits:]
                    for s in range(0, len(head), max_waits):
                        nop = mybir.InstNoOp(
                            name=f"I-waitsplit-{_waitsplit_ctr[0]}", ins=[], outs=[])
                        _waitsplit_ctr[0] += 1
                        nop.engine = inst.engine
                        nop.sync_info = mybir.SyncInfo(
                            on_wait=list(head[s:s + max_waits]), on_update=[])
                        new.append(nop)
                    inst.sync_info = mybir.SyncInfo(
                        on_wait=list(tail), on_update=list(si.on_update))
                new.append(inst)
            blk.instructions[:] = new


def _build_nc(debug=False):
    nc = bass.Bass()
    f32 = dt.float32
    GT_d = nc.declare_dram_parameter("GT", [K, N, N], BF, isOutput=False)
    XN_d = nc.declare_dram_parameter("XN", [N, BL * T * C], BF, isOutput=False)
    HN0_d = nc.declare_dram_parameter("HN0", [2, 128, N], BF, isOutput=False)
    HW0_d = nc.declare_dram_parameter("HW0", [2, H, 2 * N], BF, isOutput=False)
    WG0_d = nc.declare_dram_parameter("WG0", [128, 128], BF, isOutput=False)
    WG1_d = nc.declare_dram_parameter("WG1", [128, 128], BF, isOutput=False)
    WGX2_d = nc.declare_dram_parameter("WGX2", [70, 128], BF, isOutput=False)
    WU0_d = nc.declare_dram_parameter("WU0", [128, 64], BF, isOutput=False)
    WU1_d = nc.declare_dram_parameter("WU1", [128, 64], BF, isOutput=False)
    WUX2_d = nc.declare_dram_parameter("WUX2", [70, 64], BF, isOutput=False)
    BG_d = nc.declare_dram_parameter("BG", [128, 1], f32, isOutput=False)
    BU_d = nc.declare_dram_parameter("BU", [64, 1], f32, isOutput=False)
    EYE_d = nc.declare_dram_parameter("EYE", [128, 64], BF, isOutput=False)
    HOUT_d = nc.declare_dram_parameter("HOUT", [2, H, 2 * N], BF, isOutput=True)
    dbg = {}
    if debug:
        for nm, shp in [("DXA", [102, T * N]), ("DT01", [128, N]),
                        ("DAX", [70, N]), ("DZRS", [128, 2 * N]),
                        ("DCH", [128, 2 * N]), ("DHCS", [128, 2 * N]),
                        ("DHW1", [128, 2 * N]), ("DHN", [128, N]),
                        ("DCN", [128, N])]:
            dbg[nm] = nc.declare_dram_parameter(nm, shp, BF, isOutput=True)
    XAS_d = nc.dram_tensor("XAS_scratch", [K, BL * T * C, N], BF)

    with tile.TileContext(nc) as tc:
        with tc.tile_pool(name="const", bufs=1) as cst, \
             tc.tile_pool(name="t01s", bufs=3) as t01p, \
             tc.tile_pool(name="sb", bufs=3) as sbp, \
             tc.tile_pool(name="aggps", bufs=3, space="PSUM") as aggps, \
             tc.tile_pool(name="zrps", bufs=2, space="PSUM") as zrps, \
             tc.tile_pool(name="trps", bufs=1, space="PSUM") as trps:

            def load(shape, src_ap, tag, dtype=BF):
                d = cst.tile(shape, dtype, tag=tag)
                nc.sync.dma_start(d[:], src_ap)
                return d

            # ---- initial state first (unblocks the t-loop A phases) ----
            HN_s = [None, None]
            HW = [None, None]
            for p in range(2):
                hn0 = sbp.tile([128, N], BF, tag=f"hn{p}", name=f"hn0{p}")
                nc.sync.dma_start(hn0[:], HN0_d[p])
                HN_s[p] = hn0
                hw0 = sbp.tile([128, 2 * N], BF, tag=f"hw{p}", name=f"hw0{p}")
                nc.sync.dma_start(hw0[64:128, :], HW0_d[p])
                HW[p] = hw0

            # ---- constants / inputs ----
            gtj = []
            for j in range(NT):
                g = cst.tile([128, K * N], BF, tag=f"gt{j}", name=f"gt{j}")
                srcg = GT_d[:, j * 128:(j + 1) * 128, :].rearrange(
                    "k p i -> p k i")
                nc.sync.dma_start(g[:].rearrange("p (k i) -> p k i", k=K),
                                  srcg)
                gtj.append(g)
            gt = [[gtj[j][:, k * N:(k + 1) * N] for j in range(NT)]
                  for k in range(K)]
            xn = [load([128, BL * T * C], XN_d[j * 128:(j + 1) * 128, :],
                       f"xn{j}") for j in range(NT)]
            wg0 = load([128, 128], WG0_d[:], "wg0")
            wg1 = load([128, 128], WG1_d[:], "wg1")
            wgx2 = load([70, 128], WGX2_d[:], "wgx2")
            wu0 = load([128, 64], WU0_d[:], "wu0")
            wu1 = load([128, 64], WU1_d[:], "wu1")
            wux2 = load([70, 64], WUX2_d[:], "wux2")
            bg = load([128, 1], BG_d[:], "bg", f32)
            bu = load([64, 1], BU_d[:], "bu", f32)
            eye = load([128, 64], EYE_d[:], "eye")

            # static x-aggregates: row b*32 + k*2 + c, col t*512+i
            XA24 = cst.tile([(BL - 1) * 32 + C * K, T * N], BF, tag="xa24")


            # ---- x aggregation precompute ----
            # XA24 row b*32 + k*2 + c <- xas_k[(b,c,t), i] via SBUF->SBUF DMA
            xask = []
            for k in range(K):
                ps = aggps.tile([BL * T * C, N], f32, tag="agg")
                for j in range(NT):
                    nc.tensor.matmul(ps[:], xn[j][:], gt[k][j],
                                     start=(j == 0), stop=(j == NT - 1))
                xas = sbp.tile([BL * T * C, N], BF, tag=f"xas{k}",
                               name=f"xas{k}")
                nc.vector.tensor_copy(xas[:], ps[:])
                xask.append(xas)
            for b in range(BL):
                eng = nc.sync if b < 2 else nc.gpsimd
                for k in range(K):
                    for c in range(C):
                        row = b * 32 + k * 2 + c
                        eng.dma_start(
                            XA24[row:row + 1, :],
                            xask[k][b * 24 + c * T:b * 24 + (c + 1) * T, :])

            # ---- per-step phase bodies ----
            # engine rotation for PSUM->SBUF drains
            def drain(ci, dst_ap, src_ap):
                eng = (nc.vector.tensor_copy, nc.scalar.copy)[ci % 2]
                i_ = eng(dst_ap, src_ap)
                i_.ins.bass_priority = -20

            st = [dict(), dict()]

            def agg(p, t, src, dtag):
                """3-hop aggregation of node-major src; k0/k1 -> straight
                pair tiles tA/tB, k2+x -> per-batch [70, 512] ax tiles."""
                ax2 = []
                for bi in range(2):
                    b = 2 * p + bi
                    ax = t01p.tile([70, N], BF, tag=f"ax{dtag}{p}{bi}",
                                   name=f"ax{dtag}{p}{bi}")
                    i_ = nc.vector.tensor_copy(ax[64:70, :],
                                               XA24[b * 32:b * 32 + 6,
                                                    t * N:(t + 1) * N])
                    i_.ins.bass_priority = -20
                    ax2.append(ax)
                psk = {}
                for k in (2, 0, 1):
                    ps = aggps.tile([128, N], f32, tag="agg")
                    for j in range(NT):
                        nc.tensor.matmul(
                            ps[:], src[:, j * 128:(j + 1) * 128], gt[k][j],
                            start=(j == 0), stop=(j == NT - 1))
                    psk[k] = ps
                    if k == 2:
                        for bi in range(2):
                            drain(2 * p + bi, ax2[bi][0:64, :],
                                  ps[bi * 64:bi * 64 + 64, :])
                tAB = []
                for k in range(2):
                    tt = t01p.tile([128, N], BF, tag=f"{dtag}{p}{k}")
                    drain(2 * p + k, tt[:], psk[k][:])
                    tAB.append(tt)
                return tAB, ax2

            def transp(p, src, dst_tag):
                """wide [64,1024] bf16 -> node-major [128, 512] bf16."""
                trp = trps.tile([128, N], BF, tag="tr")
                for j in range(NT):
                    for bi in range(2):
                        nc.tensor.transpose(
                            trp[:, j * 128 + bi * 64:j * 128 + bi * 64 + 64],
                            src[64:128, bi * N + j * 128:bi * N + (j + 1) * 128],
                            eye[64:128, :])
                d = sbp.tile([128, N], BF, tag=f"{dst_tag}{p}")
                nc.vector.tensor_copy(d[:], trp[:])
                return d

            def ph_agg_gate(p, t):
                st[p]["t01g"] = agg(p, t, HN_s[p], "tg")

            def ph_gate_w(p, t):
                tAB, ax2 = st[p]["t01g"]
                zrs = sbp.tile([128, 2 * N], BF, tag=f"zrs{p}")
                zr = zrps.tile([128, 2 * N], f32, tag="zr")
                for bi in range(2):
                    sl = slice(bi * N, (bi + 1) * N)
                    bs = slice(bi * 64, bi * 64 + 64)
                    nc.tensor.matmul(zr[:, sl], wgx2[:], ax2[bi][:],
                                     start=True, stop=False)
                    nc.tensor.matmul(zr[:, sl], wg0[bs, :], tAB[0][bs, :],
                                     start=False, stop=False)
                    nc.tensor.matmul(zr[:, sl], wg1[bs, :], tAB[1][bs, :],
                                     start=False, stop=True)
                nc.scalar.activation(zrs[:], zr[:], AF.Sigmoid, bias=bg[:])
                zc = sbp.tile([128, 2 * N], BF, tag=f"zc{p}")
                i_ = nc.gpsimd.tensor_copy(zc[64:128, :], zrs[0:64, :])
                i_.ins.bass_priority = 5
                st[p]["zrs"], st[p]["zc"] = zrs, zc

            def ph_rt(p, t):
                zrs, zc = st[p]["zrs"], st[p]["zc"]
                ch = sbp.tile([128, 2 * N], BF, tag=f"ch{p}")
                nc.vector.tensor_tensor(ch[64:128, :], zrs[64:128, :],
                                        HW[p][64:128, :], ALU.mult)
                st[p]["CN"] = transp(p, ch, "cn")
                st[p]["ch_dbg"] = ch
                # u = h - z*h = (1-z)*h, off the critical path
                zh = sbp.tile([128, 2 * N], BF, tag=f"tmp{p}", bufs=2,
                              name=f"zh{p}")
                nc.gpsimd.tensor_tensor(zh[64:128, :], zc[64:128, :],
                                        HW[p][64:128, :], ALU.mult)
                u = sbp.tile([128, 2 * N], BF, tag=f"u{p}")
                nc.gpsimd.tensor_tensor(u[64:128, :], HW[p][64:128, :],
                                        zh[64:128, :], ALU.subtract)
                st[p]["u"] = u

            def ph_agg_cand(p, t):
                st[p]["t01u"] = agg(p, t, st[p]["CN"], "tu")

            def ph_upd_w(p, t):
                tAB, ax2 = st[p]["t01u"]
                hcs = sbp.tile([128, 2 * N], BF, tag=f"hcs{p}")
                hc = zrps.tile([128, 2 * N], f32, tag="zr")
                for bi in range(2):
                    sl = slice(bi * N, (bi + 1) * N)
                    bs = slice(bi * 64, bi * 64 + 64)
                    nc.tensor.matmul(hc[0:64, sl], wu0[bs, :], tAB[0][bs, :],
                                     start=True, stop=False)
                    nc.tensor.matmul(hc[0:64, sl], wu1[bs, :], tAB[1][bs, :],
                                     start=False, stop=False)
                    nc.tensor.matmul(hc[0:64, sl], wux2[:], ax2[bi][:],
                                     start=False, stop=True)
                nc.scalar.activation(hcs[64:128, :], hc[0:64, :], AF.Tanh,
                                     bias=bu[:])
                st[p]["hcs"] = hcs

            def ph_update(p, t):
                zc, hcs, u = st[p]["zc"], st[p]["hcs"], st[p]["u"]
                v = sbp.tile([128, 2 * N], BF, tag=f"tmp{p}", bufs=2)
                nc.vector.tensor_tensor(v[64:128, :], zc[64:128, :],
                                        hcs[64:128, :], ALU.mult)
                hnew = sbp.tile([128, 2 * N], BF, tag=f"hw{p}")
                nc.vector.tensor_tensor(hnew[64:128, :], u[64:128, :],
                                        v[64:128, :], ALU.add)
                HW[p] = hnew
                if t < T - 1:
                    HN_s[p] = transp(p, hnew, "hn2")
                else:
                    nc.sync.dma_start(HOUT_d[p], hnew[64:128, :])

            def dump(nm, ap):
                if debug:
                    nc.sync.dma_start(dbg[nm][0:ap.shape[0]], ap)

            def ph_dbg(p, t):
                import os
                if not debug or p != 0 or t != int(os.environ.get("DBG_T", "0")):
                    return
                dump("DXA", XA24[:])
                dump("DT01", st[0]["t01g"][0][0][:])
                dump("DAX", st[0]["t01g"][1][0][:])
                dump("DZRS", st[0]["zrs"][:])
                dump("DCH", st[0]["ch_dbg"][:])
                dump("DHCS", st[0]["hcs"][:])
                dump("DHW1", HW[0][:])
                dump("DHN", HN_s[0][:])
                dump("DCN", st[0]["CN"][:])

            PHASES = [ph_agg_gate, ph_gate_w, ph_rt, ph_agg_cand,
                      ph_upd_w, ph_update, ph_dbg]
            NPH = len(PHASES)
            OFF = 1
            for tick in range(NPH * T + OFF):
                for p in range(2):
                    local = tick - OFF * p
                    if 0 <= local < NPH * T:
                        t, ph = divmod(local, NPH)
                        PHASES[ph](p, t)

    _split_excess_waits(nc, max_waits=1)
    return nc


_NC_CACHE = {}


def _get_nc(debug=False):
    key = f"nc{debug}"
    if key not in _NC_CACHE:
        _NC_CACHE[key] = _build_nc(debug)
    return _NC_CACHE[key]


def _host_prep(G, x_seq, init_h, W_gate, b_gate, W_update, b_update):
    f32 = np.float32
    GT = np.ascontiguousarray(np.asarray(G).transpose(0, 2, 1)).astype(NPBF)
    WG3 = np.asarray(W_gate, f32).reshape(K, P, 2 * H)
    WU3 = np.asarray(W_update, f32).reshape(K, P, H)
    WG0 = np.concatenate([WG3[0, C:, :]] * 2, axis=0)
    WG1 = np.concatenate([WG3[1, C:, :]] * 2, axis=0)
    WU0 = np.concatenate([WU3[0, C:, :]] * 2, axis=0)
    WU1 = np.concatenate([WU3[1, C:, :]] * 2, axis=0)
    # x-block rows (k,c): row k*2+c = W[k, c, :]
    xg = WG3[:, :C, :].reshape(K * C, 2 * H)
    xu = WU3[:, :C, :].reshape(K * C, H)
    WGX2 = np.concatenate([WG3[2, C:, :], xg], axis=0)
    WUX2 = np.concatenate([WU3[2, C:, :], xu], axis=0)
    shared = {
        "GT": GT,
        "WG0": WG0.astype(NPBF), "WG1": WG1.astype(NPBF),
        "WGX2": WGX2.astype(NPBF),
        "WU0": WU0.astype(NPBF), "WU1": WU1.astype(NPBF),
        "WUX2": WUX2.astype(NPBF),
        "BG": np.asarray(b_gate, f32).reshape(128, 1),
        "BU": np.asarray(b_update, f32).reshape(64, 1),
        "EYE": np.concatenate([np.zeros((64, 64), f32), np.eye(64, dtype=f32)],
                      axis=0).astype(NPBF),
    }
    x_seq = np.asarray(x_seq, f32)
    init_h = np.asarray(init_h, f32)
    in_maps = []
    for c in range(NCORES):
        b0 = c * BL
        xs = x_seq[b0:b0 + BL]                     # [4, 12, 512, 2]
        h0 = init_h[b0:b0 + BL]                    # [4, 512, 64]
        m = dict(shared)
        # XN cols (b, c, t)
        m["XN"] = np.ascontiguousarray(
            xs.transpose(2, 0, 3, 1)).reshape(N, BL * T * C).astype(NPBF)
        # HN0[p][n_loc, j*128 + b*64 + f] = h0[2p+b, j*128+n_loc, f]
        hn = h0.reshape(2, 2, NT, 128, H)          # [p, b, j, n, f]
        m["HN0"] = np.ascontiguousarray(
            hn.transpose(0, 3, 2, 1, 4)).reshape(2, 128, N).astype(NPBF)
        # HW0[p][f, b*512 + i] = h0[2p+b, i, f]
        hw = h0.reshape(2, 2, N, H)                # [p, b, i, f]
        m["HW0"] = np.ascontiguousarray(
            hw.transpose(0, 3, 1, 2)).reshape(2, H, 2 * N).astype(NPBF)
        in_maps.append(m)
    return in_maps


def _run(inputs, trace=False, debug=False):
    nc = _get_nc(debug)
    in_maps = _host_prep(**inputs)
    res = run_bass_kernel_spmd(nc, in_maps, list(range(NCORES)), trace=trace)
    outs = []
    for c in range(NCORES):
        hout = np.asarray(res.results[c]["HOUT"], dtype=np.float32)
        # [2, 64, 1024] -> [4, 512, 64]
        hout = hout.reshape(2, H, 2, N).transpose(0, 2, 3, 1).reshape(
            BL, N, H)
        outs.append(hout)
    full = np.concatenate(outs, axis=0).astype(np.float32)
    return full, res


def kernel(G, x_seq, init_h, W_gate, b_gate, W_update, b_update):
    full, _ = _run(dict(G=G, x_seq=x_seq, init_h=init_h, W_gate=W_gate,
                        b_gate=b_gate, W_update=W_update, b_update=b_update))
    return full
